# revision 1
# baseline (speedup 1.0000x reference)
"""Trainium2 Bass kernel for supervised-contrastive loss (nn_ContrastiveLoss).

loss = mean over positive pairs (i,j) of (lse_i - sim_ij), where
  sim = P @ P.T / TEMP, positives = same affordance_id & different instance_id,
  lse_i = logsumexp over j != i of sim[i, :].

Decomposition
-------------
  total = sum_i n_pos_i * lse_i  -  sum_pos sim_ij
The second term is linear in sim, so it factors through class/group sums:
  sum_{aff equal}  sim_ij = sum_k ||W_k||^2 / TEMP,  W_k = sum_{aff_j=k} p_j
  sum_{code equal} sim_ij = sum_g ||G_g||^2 / TEMP,  G_g = sum_{code_j=g} p_j
  (code = (aff, inst) pair; both include the diagonal, difference removes it)
That's O(B*D) host work. The only O(B^2) quantity is lse_i, computed on
device, data-parallel over rows across 8 cores:

  per core: rows = 1024-row block; stream col-chunks [128, 1024] of
  sim = PR^T @ PT through PSUM (bf16 matmul, fp32 accum); the self column
  is masked by one extra N=128 matmul adding -BIG*I from a per-core slot
  input (slot q is -BIG*I iff chunk q holds this core's diagonal); then
    DVE  tensor_reduce(max, negate=True)        -> -rowmax
    ACT  activation(Exp, bias=-max, accum_out)  -> rowsum(exp(x - max))
  emit per (row-tile, chunk): (-max, sumexp); host merges chunks in f64.
"""

import sys

sys.path.insert(0, "/opt/trn_rl_repo")

import numpy as np
import ml_dtypes

TEMP = 0.07
B, D = 8192, 256
NCORES = 8
RPC = B // NCORES  # rows per core = 1024
NRT = RPC // 128  # row tiles per core = 8
NKH = D // 128  # contraction halves = 2
CHW = 1024  # col-chunk width (2 PSUM banks)
NCH = B // CHW  # chunks per row = 8
NMM = CHW // 512  # matmuls of N=512 per chunk half = 2
NEGBIG = -3.0e38

_cache = {}


def _build():
    """Build + compile the SPMD Bass program (same NEFF for all 8 cores)."""
    import concourse.bacc as bacc
    import concourse.tile as tile
    from concourse import mybir
    from contextlib import ExitStack

    dt = mybir.dt
    nc = bacc.Bacc("TRN2", debug=False, target_bir_lowering=False)

    pt_d = nc.dram_tensor("pt", [NKH, 128, B], dt.bfloat16, kind="ExternalInput").ap()
    pr_d = nc.dram_tensor("pr", [NKH, 128, RPC], dt.bfloat16, kind="ExternalInput").ap()
    # slots 0..NCH-1: -BIG*I iff chunk == this core's diag chunk, else 0; slot NCH: I
    dg_d = nc.dram_tensor("dg", [NCH + 1, 128, 128], dt.bfloat16, kind="ExternalInput").ap()
    st_d = nc.dram_tensor("st", [NRT, 128, 2 * NCH], dt.float32, kind="ExternalOutput").ap()

    with ExitStack() as ctx:
        tc = ctx.enter_context(tile.TileContext(nc))
        singles = ctx.enter_context(tc.tile_pool(name="singles", bufs=1))
        stats_p = ctx.enter_context(tc.tile_pool(name="stats", bufs=4))
        psum_p = ctx.enter_context(tc.tile_pool(name="ps", bufs=4, space="PSUM"))

        # DMA order matters: first chunk's operands first so PE starts early
        pr_t = [
            singles.tile([128, RPC], dt.bfloat16, tag=f"pr{h}", name=f"pr{h}")
            for h in range(NKH)
        ]
        for h in range(NKH):
            nc.sync.dma_start(out=pr_t[h], in_=pr_d[h])
        dg_t = [
            singles.tile([128, 128], dt.bfloat16, tag=f"dg{s}", name=f"dg{s}")
            for s in range(NCH + 1)
        ]
        for s in range(NCH + 1):
            nc.sync.dma_start(out=dg_t[s], in_=dg_d[s])
        ident = dg_t[NCH]
        pt_t = [
            [
                singles.tile([128, CHW], dt.bfloat16, tag=f"pt{h}c{q}", name=f"pt{h}c{q}")
                for q in range(NCH)
            ]
            for h in range(NKH)
        ]
        for q in range(NCH):
            for h in range(NKH):
                nc.sync.dma_start(out=pt_t[h][q], in_=pt_d[h, :, q * CHW : (q + 1) * CHW])

        for r in range(NRT):
            stats = stats_p.tile([128, 2 * NCH], dt.float32, tag="st")
            lhs = [pr_t[h][:, r * 128 : (r + 1) * 128] for h in range(NKH)]
            for q in range(NCH):
                ps = psum_p.tile([128, CHW], dt.float32, tag="q")
                for n in range(NMM):
                    nc.tensor.matmul(
                        ps[:, n * 512 : (n + 1) * 512],
                        lhsT=lhs[0],
                        rhs=pt_t[0][q][:, n * 512 : (n + 1) * 512],
                        start=True,
                        stop=False,
                    )
                # self-mask: adds -BIG at column (own row) iff q is the diag chunk
                nc.tensor.matmul(
                    ps[:, r * 128 : (r + 1) * 128],
                    lhsT=ident,
                    rhs=dg_t[q],
                    start=False,
                    stop=False,
                    skip_group_check=True,
                )
                for n in range(NMM):
                    nc.tensor.matmul(
                        ps[:, n * 512 : (n + 1) * 512],
                        lhsT=lhs[1],
                        rhs=pt_t[1][q][:, n * 512 : (n + 1) * 512],
                        start=False,
                        stop=True,
                    )
                nc.vector.tensor_reduce(
                    out=stats[:, q : q + 1],
                    in_=ps,
                    axis=mybir.AxisListType.X,
                    op=mybir.AluOpType.max,
                    negate=True,
                )
                nc.scalar.activation(
                    out=ps,
                    in_=ps,
                    func=mybir.ActivationFunctionType.Exp,
                    bias=stats[:, q : q + 1],
                    scale=1.0,
                    accum_out=stats[:, NCH + q : NCH + q + 1],
                )
            nc.sync.dma_start(out=st_d[r], in_=stats)

    nc.compile()
    return nc


def _get_nc():
    if "nc" not in _cache:
        _cache["nc"] = _build()
    return _cache["nc"]


def _host_prep(P):
    """Shared (all-core) device inputs + f64 copies for host-side terms."""
    s = 1.0 / np.sqrt(TEMP)
    Pd = P.astype(np.float64) * s  # scaled so sim = Pd @ Pd.T includes 1/TEMP
    Pbf = Pd.astype(ml_dtypes.bfloat16)
    # pt[h, d, j] = Pbf[j, h*128 + d]
    pt = np.ascontiguousarray(Pbf.T.reshape(NKH, 128, B))
    return Pd, Pbf, pt


def _core_inputs(c, Pbf, pt):
    rows = slice(c * RPC, (c + 1) * RPC)
    pr = np.ascontiguousarray(Pbf[rows].T.reshape(NKH, 128, RPC))
    dg = np.zeros((NCH + 1, 128, 128), ml_dtypes.bfloat16)
    eye = np.eye(128)
    qstar = c * RPC // CHW  # chunk containing this core's diagonal block
    dg[qstar] = (NEGBIG * eye).astype(ml_dtypes.bfloat16)
    dg[NCH] = eye.astype(ml_dtypes.bfloat16)
    return {"pt": pt, "pr": pr, "dg": dg}


def _lse_from_stats(st):
    """st: [NRT, 128, 2*NCH] f32 -> lse [RPC] f64 (chunk-wise stable merge)."""
    st = st.astype(np.float64)
    m_q = -st[..., :NCH]  # [NRT, 128, NCH] per-chunk row max
    s_q = st[..., NCH:]  # per-chunk sum of exp(x - m_q)
    m = m_q.max(axis=-1)
    S = (s_q * np.exp(m_q - m[..., None])).sum(axis=-1)
    return (m + np.log(S)).reshape(RPC)


def kernel(projections, affordance_ids, instance_ids):
    from concourse import bass_utils

    P = np.asarray(projections, dtype=np.float32)
    aff = np.asarray(affordance_ids).astype(np.int64)
    inst = np.asarray(instance_ids).astype(np.int64)

    Pd, Pbf, pt = _host_prep(P)
    nc = _get_nc()
    in_maps = [_core_inputs(c, Pbf, pt) for c in range(NCORES)]
    res = bass_utils.run_bass_kernel_spmd(nc, in_maps, core_ids=list(range(NCORES)))

    lse = np.concatenate([_lse_from_stats(res.results[c]["st"]) for c in range(NCORES)])

    # host-side linear terms (exact, O(B*D))
    n_aff = np.bincount(aff, minlength=16)[aff]  # |{j: aff_j = aff_i}| incl. self
    code = aff * 4096 + inst
    ucodes, inv, ccnt = np.unique(code, return_inverse=True, return_counts=True)
    n_code = ccnt[inv]  # |{j: code_j = code_i}| incl. self
    n_pos = n_aff - n_code
    N_pos = int(n_pos.sum())
    if N_pos == 0:
        return np.float32(0.0)

    W = np.zeros((16, D), np.float64)
    np.add.at(W, aff, Pd)
    T_sum = float((W * W).sum())  # sum over aff-equal ordered pairs of sim_ij
    G = np.zeros((len(ucodes), D), np.float64)
    np.add.at(G, inv, Pd)
    U_sum = float((G * G).sum())  # sum over code-equal ordered pairs of sim_ij

    total = float((n_pos * lse).sum()) - T_sum + U_sum
    return np.asarray(total / N_pos, dtype=np.float32)

